# revision 30
# baseline (speedup 1.0000x reference)
"""Trainium2 Bass kernel for nn_ContrastiveLoss (B=4096, D=512, F=128), 8 NeuronCores.

Strategy (row-sharded, per sharding hint): core c owns rows [c*512, (c+1)*512).
Host passes each core a column-rolled, transposed copy of the inputs so the
core's own rows are always local columns 0:512 (static NEFF, no per-core code).
similarity_features are L2-normalized on HOST (trivial prep, removes the whole
on-device rsqrt chain and its startup serialization).

Per core, fully fused on device (S and tsim never touch HBM), per [128 x 1024]
tile (16 tiles = 4 row-blocks x 4 col-chunks):
  S_raw = E_loc @ E^T        (PE, bf16 -> fp32 PSUM, 4 k-chunks)
  G     = f_loc @ f^T        (PE)
  variant A (10 tiles, ACT+DVE):
    sgn = Sign(G - 0.5)              (ACT, accum -> Ssgn = 2C-1024)
    m   = sgn * S_raw                (DVE STT 1x, bf16 out)
  variant B (6 tiles, DVE-only; balances engine load):
    b2  = (G > 0.5) * 2              (DVE TS 1x, accum -> 2C)
    m   = (b2 - 1) * S_raw           (DVE STT 1x)   [b2-1 = +-1 = sgn]
  both (single-src bf16 SBUF tensor_scalar -> DVE 4x mode, ~0.33us each):
    na  = m & 0x7fff = |m|           (uint16 bitcast; abs has no TS ALU op)
    sn  = min(m, 0), accum -> SN     (out discarded)
Per row-block (fat tiles amortize ACT's per-instruction overhead; pieces are
spread across the timeline and the tail piece is only 1024 wide):
  e = Exp(-10 * na)                  (ACT bf16)
  l = Ln(e + 1)                      (ACT, accum -> LL)
Row BCE sum over all j (diagonal contributes exactly 0):
  sum_j softplus(-S*sgn) = -10*SN + LL      [T = 0.1 -> scale 10]
Host combines per-row stats: row_loss, validity, final scalar.

Sign/Exp/Ln/min/mult all live in the natural_log_exp_and_others ACT table set
-> no mid-kernel table switches. (Softplus LUT is absent in this build: the
softplus_and_others set's anchor is overlaid by act2.)

This walrus build caps sync waits at 1 per instruction; _split_multiwaits
legalizes the Tile-emitted BIR by hoisting extra waits onto single-wait Drains.
"""

import json
import ml_dtypes
import numpy as np
from contextlib import ExitStack

import concourse.bass as bass
import concourse.tile as tile
import concourse.mybir as mybir
from concourse.bass_utils import run_bass_kernel_spmd

f32 = mybir.dt.float32
bf16 = mybir.dt.bfloat16
u16 = mybir.dt.uint16
AFT = mybir.ActivationFunctionType
ALU = mybir.AluOpType

B, D, F = 4096, 512, 128
NCORES = 8
RPC = B // NCORES          # 512 rows per core
NR = RPC // 128            # 4 row blocks of 128
CHUNK = 1024               # column chunk (2 PSUM banks)
NN = B // CHUNK            # 4 column chunks
NT = NR * NN               # 16 stat columns
KC = D // 128              # 4 contraction chunks
INV_T = 10.0               # 1/TEMPERATURE
# DVE-only tiles (variant B): 6 total.
# Balances ACT (Sign+Exp+Ln) against DVE (b2/m/na/sn) engine time.
B_TILES = frozenset([(0, 1), (0, 3), (1, 1), (2, 1), (2, 3), (3, 1)])
# Exp/Ln pieces: (row_block, col_lo, col_hi, ll_col). Spread across the
# timeline at half-row-block granularity; the tail is a single 1024 piece.
LL_PIECES = [
    (0, 0, 2048, 0), (0, 2048, 4096, 1),
    (1, 0, 2048, 2), (1, 2048, 4096, 3),
    (2, 0, 2048, 4), (2, 2048, 4096, 5),
    (3, 0, 2048, 6), (3, 2048, 3072, 7), (3, 3072, 4096, 8),
]
NLL = len(LL_PIECES)
# merged stats layout in the single output tensor
SN_OFF, C_OFF, LL_OFF = 0, 16, 32
ST_W = LL_OFF + NLL


def _split_multiwaits(m: dict) -> int:
    """Split >1-wait instructions into single-wait Drain chains (walrus cap)."""
    n_new = 0
    for fn in m["functions"]:
        for blk in fn["blocks"]:
            out = []
            for inst in blk["instructions"]:
                si = inst.get("sync_info") or {}
                ow = si.get("on_wait") or []
                if len(ow) > 1:
                    for w in ow[:-1]:
                        n_new += 1
                        out.append({
                            "debug": inst.get("debug", 0),
                            "engine": inst["engine"],
                            "ins": [], "outs": [],
                            "is_reset_sema": False,
                            "name": f"{inst['name']}-sw{n_new}",
                            "opcode": "Drain",
                            "sync_info": {"on_update": [], "on_wait": [w]},
                        })
                    si["on_wait"] = [ow[-1]]
                out.append(inst)
            blk["instructions"] = out
    return n_new


def _build_nc() -> bass.Bass:
    nc = bass.Bass("TRN2", target_bir_lowering=False, debug=False)
    et_d = nc.dram_tensor("et", [D, B], bf16, kind="ExternalInput").ap()
    sfn_d = nc.dram_tensor("sfn", [F, B], bf16, kind="ExternalInput").ap()
    # single merged stats output: [ab(16) | m(16) | c(16) | ll(5)]; the last
    # row-block's Ln runs in two pieces, each with its own accum column
    # (accum_out overwrites, it does not accumulate)
    out_st = nc.dram_tensor("out_st", [128, ST_W], f32,
                            kind="ExternalOutput").ap()

    with tile.TileContext(nc) as tc, ExitStack() as ctx:
        main = ctx.enter_context(tc.tile_pool(name="main", bufs=1))
        scratch = ctx.enter_context(tc.tile_pool(name="scratch", bufs=3))
        fat = ctx.enter_context(tc.tile_pool(name="fat", bufs=2))

        # DMA doorbells cost ~0.6us each on Sync and issue sequentially, so
        # ring the critical pieces first: et n4=0 (first S matmul), then sfn
        # as ONE dma (first G), then the remaining et pieces.
        sfn_sb = main.tile([F, B], bf16, name="sfn_sb")
        et_sb = [main.tile([128, B], bf16, name=f"et{kc}") for kc in range(KC)]
        for kc in range(KC):
            nc.sync.dma_start(out=et_sb[kc][:, 0:CHUNK],
                              in_=et_d[kc * 128:(kc + 1) * 128, 0:CHUNK])
        nc.sync.dma_start(out=sfn_sb, in_=sfn_d)
        for n4 in range(1, NN):
            for kc in range(KC):
                c0 = n4 * CHUNK
                nc.sync.dma_start(
                    out=et_sb[kc][:, c0:c0 + CHUNK],
                    in_=et_d[kc * 128:(kc + 1) * 128, c0:c0 + CHUNK])

        neg_half = main.tile([128, 1], f32, name="neg_half")
        nc.vector.memset(neg_half, -0.5)

        st = main.tile([128, ST_W], f32, name="st")
        sn_st = st[:, SN_OFF:SN_OFF + NT]
        c_st = st[:, C_OFF:C_OFF + NT]
        ll_st = st[:, LL_OFF:LL_OFF + NLL]

        # --- main loop over 16 tiles [128 rows x 1024 cols] ---
        with tc.tile_pool(name="pp_s", bufs=2, space="PSUM") as pp_s, \
             tc.tile_pool(name="pp_g", bufs=2, space="PSUM") as pp_g:
            na_fat = {}

            def act_expln(pr, pc0, pc1, ll_col):
                """Exp then Ln over na[pr][:, pc0:pc1], Ln accum to ll col."""
                na_t = na_fat[pr]
                e_t = fat.tile([128, B], bf16, name="e_t", bufs=2)
                nc.scalar.activation(e_t[:, pc0:pc1], na_t[:, pc0:pc1],
                                     AFT.Exp, scale=-INV_T)
                l_t = fat.tile([128, B], bf16, name="l_t", bufs=2)
                nc.scalar.activation(l_t[:, pc0:pc1], e_t[:, pc0:pc1],
                                     AFT.Ln, bias=1.0,
                                     accum_out=ll_st[:, ll_col:ll_col + 1])

            # piece (r', lo, hi) is runnable once na chunks [0, hi) of row
            # block r' exist; schedule each piece at the earliest (r, n4)
            # AFTER its last na chunk, spreading ACT load evenly.
            piece_at = {}  # (r, n4) -> list of LL_PIECES entries
            for pr, lo, hi, col in LL_PIECES:
                last_chunk = (hi - 1) // CHUNK
                r_at, n_at = pr, last_chunk
                piece_at.setdefault((r_at, n_at), []).append((pr, lo, hi, col))

            for r in range(NR):
                na_fat[r] = fat.tile([128, B], bf16, name=f"na{r % 2}")
                for n4 in range(NN):
                    idx = r * NN + n4
                    c0 = n4 * CHUNK
                    psS = pp_s.tile([128, CHUNK], f32, name="psS")
                    for kc in range(KC):
                        for h in range(2):
                            nc.tensor.matmul(
                                psS[:, h * 512:(h + 1) * 512],
                                et_sb[kc][:, r * 128:(r + 1) * 128],
                                et_sb[kc][:, c0 + h * 512:c0 + (h + 1) * 512],
                                start=(kc == 0), stop=(kc == KC - 1))
                    psG = pp_g.tile([128, CHUNK], f32, name="psG")
                    for h in range(2):
                        nc.tensor.matmul(
                            psG[:, h * 512:(h + 1) * 512],
                            sfn_sb[:, r * 128:(r + 1) * 128],
                            sfn_sb[:, c0 + h * 512:c0 + (h + 1) * 512],
                            start=True, stop=True)

                    if (r, n4) in B_TILES:
                        # variant B: compare+scale on DVE, no ACT use
                        b2 = scratch.tile([128, CHUNK], bf16, name="b2")
                        nc.vector.tensor_scalar(
                            out=b2, in0=psG, scalar1=0.5, scalar2=2.0,
                            op0=ALU.is_gt, op1=ALU.mult,
                            accum_out=c_st[:, idx:idx + 1])
                        m_t = scratch.tile([128, CHUNK], bf16, name="m_t")
                        nc.vector.scalar_tensor_tensor(
                            out=m_t, in0=b2, scalar=-1.0, in1=psS,
                            op0=ALU.add, op1=ALU.mult)
                    else:
                        # variant A: sign on ACT
                        sgn_t = scratch.tile([128, CHUNK], bf16, name="sgn_t")
                        nc.scalar.activation(sgn_t, psG, AFT.Sign,
                                             bias=neg_half,
                                             accum_out=c_st[:, idx:idx + 1])
                        m_t = scratch.tile([128, CHUNK], bf16, name="m_t")
                        nc.vector.scalar_tensor_tensor(
                            out=m_t, in0=sgn_t, scalar=1.0, in1=psS,
                            op0=ALU.mult, op1=ALU.mult)
                    # na = |m| via bf16 sign-bit clear: single-src uint16 SBUF
                    # tensor_scalar -> DVE 4x mode (abs has no TS ALU op)
                    nc.vector.tensor_scalar(
                        out=na_fat[r][:, c0:c0 + CHUNK].bitcast(u16),
                        in0=m_t.bitcast(u16), scalar1=0x7FFF, scalar2=0,
                        op0=ALU.bitwise_and, op1=ALU.bitwise_or)
                    # sn accum: sum(min(m,0)); row bce = -10*SN + LL
                    dump = scratch.tile([128, CHUNK], bf16, name="dump")
                    nc.vector.tensor_scalar(
                        out=dump, in0=m_t, scalar1=0.0, scalar2=0.0,
                        op0=ALU.min, op1=ALU.add,
                        accum_out=sn_st[:, idx:idx + 1])
                    for piece in piece_at.get((r, n4), []):
                        act_expln(*piece)

        nc.sync.dma_start(out=out_st, in_=st)

    orig = nc.to_json_bytes

    def patched():
        m = json.loads(orig())
        _split_multiwaits(m)
        return json.dumps(m).encode()

    nc.to_json_bytes = patched
    return nc


_NC_CACHE = None
last_run = None  # BassKernelResults of the most recent kernel() call


def _get_nc():
    global _NC_CACHE
    if _NC_CACHE is None:
        _NC_CACHE = _build_nc()
    return _NC_CACHE


def kernel(embeddings: np.ndarray, similarity_features: np.ndarray) -> np.ndarray:
    global last_run
    E = np.asarray(embeddings, dtype=np.float32)
    SF = np.asarray(similarity_features, dtype=np.float32)
    assert E.shape == (B, D) and SF.shape == (B, F)

    ET = E.T.astype(ml_dtypes.bfloat16)    # [D, B]
    fn = SF.astype(np.float64)
    fn = fn / np.maximum(np.linalg.norm(fn, axis=1, keepdims=True), 1e-12)
    SFNT = fn.T.astype(ml_dtypes.bfloat16)  # [F, B] normalized
    in_maps = []
    for c in range(NCORES):
        sh = c * RPC
        in_maps.append({
            "et": np.ascontiguousarray(np.roll(ET, -sh, axis=1)),
            "sfn": np.ascontiguousarray(np.roll(SFNT, -sh, axis=1)),
        })

    nc = _get_nc()
    res = run_bass_kernel_spmd(nc, in_maps, core_ids=list(range(NCORES)))
    last_run = res

    # host combine: [p, r*NN+n] stats -> per-row scalars (order within a core:
    # local row i = r*128 + p; global row = c*RPC + i, irrelevant to the sums)
    bce_num = np.zeros((NCORES, RPC), np.float64)
    pos_all = np.zeros((NCORES, RPC), np.float64)
    is_b = np.array([[(r, n) in B_TILES for n in range(NN)] for r in range(NR)])
    for c, r in enumerate(res.results):
        stt = r["out_st"].astype(np.float64)
        sn = stt[:, SN_OFF:SN_OFF + NT].reshape(128, NR, NN).sum(axis=2)
        llp = stt[:, LL_OFF:LL_OFF + NLL]
        ll = np.zeros((128, NR))
        for (pr, _, _, col) in LL_PIECES:
            ll[:, pr] += llp[:, col]
        cc = stt[:, C_OFF:C_OFF + NT].reshape(128, NR, NN)
        # variant A columns hold sum(sgn) = 2*pos - 1024; variant B hold 2*pos
        pos = np.where(is_b[None, :, :], cc / 2.0, (cc + CHUNK) / 2.0).sum(axis=2)
        bce = -INV_T * sn + ll                       # [128, NR]
        bce_num[c] = bce.T.reshape(RPC)              # index r*128+p
        pos_all[c] = pos.T.reshape(RPC)

    bce_num = bce_num.reshape(-1)
    pos_all = pos_all.reshape(-1)
    row_loss = bce_num / np.float64(B - 1)
    pos_off = pos_all - 1.0                 # drop the diagonal positive
    neg_off = (B - 1) - pos_off
    valid = (pos_off >= 0.5) & (neg_off >= 0.5)
    num_valid = max(int(valid.sum()), 1)
    loss = np.float64(np.sum(np.where(valid, row_loss, 0.0))) / num_valid
    return np.float32(loss)


# revision 39
# speedup vs baseline: 1.0266x; 1.0266x over previous
"""Trainium2 Bass kernel for nn_ContrastiveLoss (B=4096, D=512, F=128), 8 NeuronCores.

Strategy (row-sharded, per sharding hint): core c owns rows [c*512, (c+1)*512).
Host passes each core a column-rolled, transposed copy of the inputs so the
core's own rows are always local columns 0:512 (static NEFF, no per-core code).
similarity_features are L2-normalized on HOST (trivial prep, removes the whole
on-device rsqrt chain and its startup serialization).

Per core, fully fused on device (S and tsim never touch HBM), per [128 x 1024]
tile (16 tiles = 4 row-blocks x 4 col-chunks):
  S_raw = E_loc @ E^T        (PE, bf16 -> fp32 PSUM, 4 k-chunks)
  G     = f_loc @ f^T        (PE)
  variant A (8 tiles, ACT+DVE):
    sgn = Sign(G - 0.5)              (ACT, accum -> Ssgn = 2C-1024)
    m   = sgn * S_raw                (DVE STT 1x, accum -> M)
  variant B (8 tiles, DVE-only; balances engine load):
    b2  = (G > 0.5) * 2              (DVE TS 1x, accum -> 2C)
    m   = (b2 - 1) * S_raw           (DVE STT 1x, accum -> M)  [b2-1 = sgn]
  both:
    na  = min(-m, m) = -|m|          (DVE STT, accum -> -AB; accumulating
                                      DVE ops always run 1x, so one fused
                                      STT beats any multi-pass split)
Per row-block (fat tiles amortize ACT's per-instruction overhead; pieces are
scheduled at the earliest point their na chunks exist, and the last
row-block is chunked so the tail after the final matmul is ~1 chunk):
  e = Exp(10 * na)                   (ACT bf16)
  l = Ln(e + 1)                      (ACT, accum -> LL)
Row BCE sum over all j (diagonal contributes exactly 0):
  sum_j softplus(-S*sgn) = 5*(AB - M) + LL      [T = 0.1 -> scale 10]
Host combines per-row stats: row_loss, validity, final scalar.

Sign/Exp/Ln/min/mult all live in the natural_log_exp_and_others ACT table set
-> no mid-kernel table switches. (Softplus LUT is absent in this build: the
softplus_and_others set's anchor is overlaid by act2.)

This walrus build caps sync waits at 1 per instruction; _split_multiwaits
legalizes the Tile-emitted BIR by hoisting extra waits onto single-wait Drains.
"""

import json
import ml_dtypes
import numpy as np
from contextlib import ExitStack

import concourse.bass as bass
import concourse.tile as tile
import concourse.mybir as mybir
from concourse.bass_utils import run_bass_kernel_spmd

f32 = mybir.dt.float32
bf16 = mybir.dt.bfloat16
u16 = mybir.dt.uint16
AFT = mybir.ActivationFunctionType
ALU = mybir.AluOpType

B, D, F = 4096, 512, 128
NCORES = 8
RPC = B // NCORES          # 512 rows per core
NR = RPC // 128            # 4 row blocks of 128
CHUNK = 1024               # column chunk (2 PSUM banks)
NN = B // CHUNK            # 4 column chunks
NT = NR * NN               # 16 stat columns
KC = D // 128              # 4 contraction chunks
INV_T = 10.0               # 1/TEMPERATURE
# DVE-only tiles (variant B): 2 per row-block -> 8 total.
# Balances ACT (Sign+Exp+Ln) against DVE (b2/m/na) engine time.
B_TILES = frozenset((r, n) for r in range(4) for n in (1, 3))
# Exp/Ln pieces: (row_block, col_lo, col_hi, ll_col). Fat 4096-wide pieces
# amortize ACT per-instruction overhead; the last row-block is chunked so
# only a 1024-wide piece remains after the final na.
LL_PIECES = [
    (0, 0, 4096, 0),
    (1, 0, 4096, 1),
    (2, 0, 4096, 2),
    (3, 0, 2048, 3), (3, 2048, 3072, 4), (3, 3072, 4096, 5),
]
NLL = len(LL_PIECES)
# merged stats layout in the single output tensor
AB_OFF, M_OFF, C_OFF, LL_OFF = 0, 16, 32, 48
ST_W = LL_OFF + NLL


def _split_multiwaits(m: dict) -> int:
    """Split >1-wait instructions into single-wait Drain chains (walrus cap)."""
    n_new = 0
    for fn in m["functions"]:
        for blk in fn["blocks"]:
            out = []
            for inst in blk["instructions"]:
                si = inst.get("sync_info") or {}
                ow = si.get("on_wait") or []
                if len(ow) > 1:
                    for w in ow[:-1]:
                        n_new += 1
                        out.append({
                            "debug": inst.get("debug", 0),
                            "engine": inst["engine"],
                            "ins": [], "outs": [],
                            "is_reset_sema": False,
                            "name": f"{inst['name']}-sw{n_new}",
                            "opcode": "Drain",
                            "sync_info": {"on_update": [], "on_wait": [w]},
                        })
                    si["on_wait"] = [ow[-1]]
                out.append(inst)
            blk["instructions"] = out
    return n_new


def _build_nc() -> bass.Bass:
    nc = bass.Bass("TRN2", target_bir_lowering=False, debug=False)
    et_d = nc.dram_tensor("et", [D, B], bf16, kind="ExternalInput").ap()
    sfn_d = nc.dram_tensor("sfn", [F, B], bf16, kind="ExternalInput").ap()
    # single merged stats output: [ab(16) | m(16) | c(16) | ll(5)]; the last
    # row-block's Ln runs in two pieces, each with its own accum column
    # (accum_out overwrites, it does not accumulate)
    out_st = nc.dram_tensor("out_st", [128, ST_W], f32,
                            kind="ExternalOutput").ap()

    with tile.TileContext(nc) as tc, ExitStack() as ctx:
        main = ctx.enter_context(tc.tile_pool(name="main", bufs=1))
        scratch = ctx.enter_context(tc.tile_pool(name="scratch", bufs=3))
        fat = ctx.enter_context(tc.tile_pool(name="fat", bufs=2))

        # DMA doorbells cost ~0.6us each on Sync and issue sequentially, so
        # ring the critical pieces first: et n4=0 (first S matmul), then sfn
        # as ONE dma (first G), then the remaining et pieces.
        sfn_sb = main.tile([F, B], bf16, name="sfn_sb")
        et_sb = [main.tile([128, B], bf16, name=f"et{kc}") for kc in range(KC)]
        for kc in range(KC):
            nc.sync.dma_start(out=et_sb[kc][:, 0:CHUNK],
                              in_=et_d[kc * 128:(kc + 1) * 128, 0:CHUNK])
        nc.sync.dma_start(out=sfn_sb, in_=sfn_d)
        for n4 in range(1, NN):
            for kc in range(KC):
                c0 = n4 * CHUNK
                nc.sync.dma_start(
                    out=et_sb[kc][:, c0:c0 + CHUNK],
                    in_=et_d[kc * 128:(kc + 1) * 128, c0:c0 + CHUNK])

        neg_half = main.tile([128, 1], f32, name="neg_half")
        nc.vector.memset(neg_half, -0.5)

        st = main.tile([128, ST_W], f32, name="st")
        ab_st = st[:, AB_OFF:AB_OFF + NT]
        m_st = st[:, M_OFF:M_OFF + NT]
        c_st = st[:, C_OFF:C_OFF + NT]
        ll_st = st[:, LL_OFF:LL_OFF + NLL]

        # --- main loop over 16 tiles [128 rows x 1024 cols] ---
        with tc.tile_pool(name="pp_s", bufs=2, space="PSUM") as pp_s, \
             tc.tile_pool(name="pp_g", bufs=2, space="PSUM") as pp_g:
            na_fat = {}

            def act_expln(pr, pc0, pc1, ll_col):
                """Exp then Ln over na[pr][:, pc0:pc1], Ln accum to ll col."""
                na_t = na_fat[pr]
                e_t = fat.tile([128, B], bf16, name="e_t", bufs=2)
                nc.scalar.activation(e_t[:, pc0:pc1], na_t[:, pc0:pc1],
                                     AFT.Exp, scale=INV_T)
                l_t = fat.tile([128, B], bf16, name="l_t", bufs=2)
                nc.scalar.activation(l_t[:, pc0:pc1], e_t[:, pc0:pc1],
                                     AFT.Ln, bias=1.0,
                                     accum_out=ll_st[:, ll_col:ll_col + 1])

            # piece (r', lo, hi) is runnable once na chunks [0, hi) of row
            # block r' exist; schedule each piece at the earliest (r, n4)
            # AFTER its last na chunk, spreading ACT load evenly.
            piece_at = {}  # (r, n4) -> list of LL_PIECES entries
            for pr, lo, hi, col in LL_PIECES:
                last_chunk = (hi - 1) // CHUNK
                r_at, n_at = pr, last_chunk
                piece_at.setdefault((r_at, n_at), []).append((pr, lo, hi, col))

            for r in range(NR):
                na_fat[r] = fat.tile([128, B], bf16, name=f"na{r % 2}")
                for n4 in range(NN):
                    idx = r * NN + n4
                    c0 = n4 * CHUNK
                    psS = pp_s.tile([128, CHUNK], f32, name="psS")
                    for kc in range(KC):
                        for h in range(2):
                            nc.tensor.matmul(
                                psS[:, h * 512:(h + 1) * 512],
                                et_sb[kc][:, r * 128:(r + 1) * 128],
                                et_sb[kc][:, c0 + h * 512:c0 + (h + 1) * 512],
                                start=(kc == 0), stop=(kc == KC - 1))
                    psG = pp_g.tile([128, CHUNK], f32, name="psG")
                    for h in range(2):
                        nc.tensor.matmul(
                            psG[:, h * 512:(h + 1) * 512],
                            sfn_sb[:, r * 128:(r + 1) * 128],
                            sfn_sb[:, c0 + h * 512:c0 + (h + 1) * 512],
                            start=True, stop=True)

                    if (r, n4) in B_TILES:
                        # variant B: compare+scale on DVE, no ACT use
                        b2 = scratch.tile([128, CHUNK], bf16, name="b2")
                        nc.vector.tensor_scalar(
                            out=b2, in0=psG, scalar1=0.5, scalar2=2.0,
                            op0=ALU.is_gt, op1=ALU.mult,
                            accum_out=c_st[:, idx:idx + 1])
                        m_t = scratch.tile([128, CHUNK], bf16, name="m_t")
                        nc.vector.scalar_tensor_tensor(
                            out=m_t, in0=b2, scalar=-1.0, in1=psS,
                            op0=ALU.add, op1=ALU.mult,
                            accum_out=m_st[:, idx:idx + 1])
                    else:
                        # variant A: sign on ACT
                        sgn_t = scratch.tile([128, CHUNK], bf16, name="sgn_t")
                        nc.scalar.activation(sgn_t, psG, AFT.Sign,
                                             bias=neg_half,
                                             accum_out=c_st[:, idx:idx + 1])
                        m_t = scratch.tile([128, CHUNK], bf16, name="m_t")
                        nc.vector.scalar_tensor_tensor(
                            out=m_t, in0=sgn_t, scalar=1.0, in1=psS,
                            op0=ALU.mult, op1=ALU.mult,
                            accum_out=m_st[:, idx:idx + 1])
                    # na = min(-m, m) = -|m|, accum -> -AB (accumulating DVE
                    # ops always run at 1x, so one fused STT beats any split)
                    nc.vector.scalar_tensor_tensor(
                        out=na_fat[r][:, c0:c0 + CHUNK], in0=m_t, scalar=-1.0,
                        in1=m_t, op0=ALU.mult, op1=ALU.min,
                        accum_out=ab_st[:, idx:idx + 1])
                    for piece in piece_at.get((r, n4), []):
                        act_expln(*piece)

        nc.sync.dma_start(out=out_st, in_=st)

    orig = nc.to_json_bytes

    def patched():
        m = json.loads(orig())
        _split_multiwaits(m)
        return json.dumps(m).encode()

    nc.to_json_bytes = patched
    return nc


_NC_CACHE = None
last_run = None  # BassKernelResults of the most recent kernel() call


def _get_nc():
    global _NC_CACHE
    if _NC_CACHE is None:
        _NC_CACHE = _build_nc()
    return _NC_CACHE


def kernel(embeddings: np.ndarray, similarity_features: np.ndarray) -> np.ndarray:
    global last_run
    E = np.asarray(embeddings, dtype=np.float32)
    SF = np.asarray(similarity_features, dtype=np.float32)
    assert E.shape == (B, D) and SF.shape == (B, F)

    ET = E.T.astype(ml_dtypes.bfloat16)    # [D, B]
    fn = SF.astype(np.float64)
    fn = fn / np.maximum(np.linalg.norm(fn, axis=1, keepdims=True), 1e-12)
    SFNT = fn.T.astype(ml_dtypes.bfloat16)  # [F, B] normalized
    in_maps = []
    for c in range(NCORES):
        sh = c * RPC
        in_maps.append({
            "et": np.ascontiguousarray(np.roll(ET, -sh, axis=1)),
            "sfn": np.ascontiguousarray(np.roll(SFNT, -sh, axis=1)),
        })

    nc = _get_nc()
    res = run_bass_kernel_spmd(nc, in_maps, core_ids=list(range(NCORES)))
    last_run = res

    # host combine: [p, r*NN+n] stats -> per-row scalars (order within a core:
    # local row i = r*128 + p; global row = c*RPC + i, irrelevant to the sums)
    bce_num = np.zeros((NCORES, RPC), np.float64)
    pos_all = np.zeros((NCORES, RPC), np.float64)
    is_b = np.array([[(r, n) in B_TILES for n in range(NN)] for r in range(NR)])
    for c, r in enumerate(res.results):
        stt = r["out_st"].astype(np.float64)
        ab = -stt[:, AB_OFF:AB_OFF + NT].reshape(128, NR, NN).sum(axis=2)
        m = stt[:, M_OFF:M_OFF + NT].reshape(128, NR, NN).sum(axis=2)
        llp = stt[:, LL_OFF:LL_OFF + NLL]
        ll = np.zeros((128, NR))
        for (pr, _, _, col) in LL_PIECES:
            ll[:, pr] += llp[:, col]
        cc = stt[:, C_OFF:C_OFF + NT].reshape(128, NR, NN)
        # variant A columns hold sum(sgn) = 2*pos - 1024; variant B hold 2*pos
        pos = np.where(is_b[None, :, :], cc / 2.0, (cc + CHUNK) / 2.0).sum(axis=2)
        bce = 0.5 * INV_T * (ab - m) + ll            # [128, NR]
        bce_num[c] = bce.T.reshape(RPC)              # index r*128+p
        pos_all[c] = pos.T.reshape(RPC)

    bce_num = bce_num.reshape(-1)
    pos_all = pos_all.reshape(-1)
    row_loss = bce_num / np.float64(B - 1)
    pos_off = pos_all - 1.0                 # drop the diagonal positive
    neg_off = (B - 1) - pos_off
    valid = (pos_off >= 0.5) & (neg_off >= 0.5)
    num_valid = max(int(valid.sum()), 1)
    loss = np.float64(np.sum(np.where(valid, row_loss, 0.0))) / num_valid
    return np.float32(loss)


# revision 42
# speedup vs baseline: 1.0345x; 1.0077x over previous
"""Trainium2 Bass kernel for nn_ContrastiveLoss (B=4096, D=512, F=128), 8 NeuronCores.

Strategy (row-sharded, per sharding hint): core c owns rows [c*512, (c+1)*512).
Host passes each core a column-rolled, transposed copy of the inputs so the
core's own rows are always local columns 0:512 (static NEFF, no per-core code).
similarity_features are L2-normalized on HOST (trivial prep, removes the whole
on-device rsqrt chain and its startup serialization).

Per core, fully fused on device (S and tsim never touch HBM), per [128 x 1024]
tile (16 tiles = 4 row-blocks x 4 col-chunks):
  S_raw = E_loc @ E^T        (PE, bf16 -> fp32 PSUM, 4 k-chunks)
  G     = f_loc @ f^T        (PE)
  variant A (8 tiles, ACT+DVE):
    sgn = Sign(G - 0.5)              (ACT, accum -> Ssgn = 2C-1024)
    m   = sgn * S_raw                (DVE STT 1x, accum -> M)
  variant B (8 tiles, DVE-only; balances engine load):
    b2  = (G > 0.5) * 2              (DVE TS 1x, accum -> 2C)
    m   = (b2 - 1) * S_raw           (DVE STT 1x, accum -> M)  [b2-1 = sgn]
  both:
    na  = min(-m, m) = -|m|          (DVE STT, accum -> -AB; accumulating
                                      DVE ops always run 1x, so one fused
                                      STT beats any multi-pass split)
Per row-block (fat tiles amortize ACT's per-instruction overhead; pieces are
scheduled at the earliest point their na chunks exist, and the last
row-block is chunked so the tail after the final matmul is ~1 chunk):
  e = Exp(10 * na)                   (ACT bf16)
  l = Ln(e + 1)                      (ACT, accum -> LL)
Row BCE sum over all j (diagonal contributes exactly 0):
  sum_j softplus(-S*sgn) = 5*(AB - M) + LL      [T = 0.1 -> scale 10]
Host combines per-row stats: row_loss, validity, final scalar.

Sign/Exp/Ln/min/mult all live in the natural_log_exp_and_others ACT table set
-> no mid-kernel table switches. (Softplus LUT is absent in this build: the
softplus_and_others set's anchor is overlaid by act2.)

This walrus build caps sync waits at 1 per instruction; _split_multiwaits
legalizes the Tile-emitted BIR by hoisting extra waits onto single-wait Drains.
"""

import json
import ml_dtypes
import numpy as np
from contextlib import ExitStack

import concourse.bass as bass
import concourse.tile as tile
import concourse.mybir as mybir
from concourse.bass_utils import run_bass_kernel_spmd

f32 = mybir.dt.float32
bf16 = mybir.dt.bfloat16
u16 = mybir.dt.uint16
AFT = mybir.ActivationFunctionType
ALU = mybir.AluOpType

B, D, F = 4096, 512, 128
NCORES = 8
RPC = B // NCORES          # 512 rows per core
NR = RPC // 128            # 4 row blocks of 128
CHUNK = 1024               # column chunk (2 PSUM banks)
NN = B // CHUNK            # 4 column chunks
NT = NR * NN               # 16 stat columns
KC = D // 128              # 4 contraction chunks
INV_T = 10.0               # 1/TEMPERATURE
# DVE-only tiles (variant B): 2 per row-block -> 8 total.
# Balances ACT (Sign+Exp+Ln) against DVE (b2/m/na) engine time.
B_TILES = frozenset((r, n) for r in range(4) for n in (1, 3))
# Exp/Ln pieces: (row_block, col_lo, col_hi, ll_col). Fat 4096-wide pieces
# amortize ACT per-instruction overhead; the last row-block is chunked so
# only a 1024-wide piece remains after the final na.
LL_PIECES = [
    (0, 0, 4096, 0),
    (1, 0, 4096, 1),
    (2, 0, 4096, 2),
    (3, 0, 2048, 3), (3, 2048, 3072, 4), (3, 3072, 4096, 5),
]
# issue window per piece: one window AFTER the last needed na chunk, so the
# ACT queue never head-of-line blocks on a just-issued DVE na pass
LL_SCHED = {0: (1, 0), 1: (2, 0), 2: (3, 0), 3: (3, 3), 4: None, 5: None}
NLL = len(LL_PIECES)
# merged stats layout in the single output tensor
AB_OFF, M_OFF, C_OFF, LL_OFF = 0, 16, 32, 48
ST_W = LL_OFF + NLL


def _split_multiwaits(m: dict) -> int:
    """Split >1-wait instructions into single-wait Drain chains (walrus cap)."""
    n_new = 0
    for fn in m["functions"]:
        for blk in fn["blocks"]:
            out = []
            for inst in blk["instructions"]:
                si = inst.get("sync_info") or {}
                ow = si.get("on_wait") or []
                if len(ow) > 1:
                    for w in ow[:-1]:
                        n_new += 1
                        out.append({
                            "debug": inst.get("debug", 0),
                            "engine": inst["engine"],
                            "ins": [], "outs": [],
                            "is_reset_sema": False,
                            "name": f"{inst['name']}-sw{n_new}",
                            "opcode": "Drain",
                            "sync_info": {"on_update": [], "on_wait": [w]},
                        })
                    si["on_wait"] = [ow[-1]]
                out.append(inst)
            blk["instructions"] = out
    return n_new


def _build_nc() -> bass.Bass:
    nc = bass.Bass("TRN2", target_bir_lowering=False, debug=False)
    et_d = nc.dram_tensor("et", [D, B], bf16, kind="ExternalInput").ap()
    sfn_d = nc.dram_tensor("sfn", [F, B], bf16, kind="ExternalInput").ap()
    # single merged stats output: [ab(16) | m(16) | c(16) | ll(5)]; the last
    # row-block's Ln runs in two pieces, each with its own accum column
    # (accum_out overwrites, it does not accumulate)
    out_st = nc.dram_tensor("out_st", [128, ST_W], f32,
                            kind="ExternalOutput").ap()

    with tile.TileContext(nc) as tc, ExitStack() as ctx:
        main = ctx.enter_context(tc.tile_pool(name="main", bufs=1))
        scratch = ctx.enter_context(tc.tile_pool(name="scratch", bufs=3))
        fat = ctx.enter_context(tc.tile_pool(name="fat", bufs=2))

        # DMA doorbells cost ~0.6us each on Sync and issue sequentially, so
        # ring the critical pieces first: et n4=0 (first S matmul), then sfn
        # as ONE dma (first G), then the remaining et pieces.
        sfn_sb = main.tile([F, B], bf16, name="sfn_sb")
        et_sb = [main.tile([128, B], bf16, name=f"et{kc}") for kc in range(KC)]
        for kc in range(KC):
            nc.sync.dma_start(out=et_sb[kc][:, 0:CHUNK],
                              in_=et_d[kc * 128:(kc + 1) * 128, 0:CHUNK])
        nc.sync.dma_start(out=sfn_sb, in_=sfn_d)
        for n4 in range(1, NN):
            for kc in range(KC):
                c0 = n4 * CHUNK
                nc.sync.dma_start(
                    out=et_sb[kc][:, c0:c0 + CHUNK],
                    in_=et_d[kc * 128:(kc + 1) * 128, c0:c0 + CHUNK])

        neg_half = main.tile([128, 1], f32, name="neg_half")
        nc.vector.memset(neg_half, -0.5)

        st = main.tile([128, ST_W], f32, name="st")
        ab_st = st[:, AB_OFF:AB_OFF + NT]
        m_st = st[:, M_OFF:M_OFF + NT]
        c_st = st[:, C_OFF:C_OFF + NT]
        ll_st = st[:, LL_OFF:LL_OFF + NLL]

        # --- main loop over 16 tiles [128 rows x 1024 cols] ---
        with tc.tile_pool(name="pp_s", bufs=2, space="PSUM") as pp_s, \
             tc.tile_pool(name="pp_g", bufs=2, space="PSUM") as pp_g:
            na_fat = {}

            def act_expln(pr, pc0, pc1, ll_col):
                """Exp then Ln over na[pr][:, pc0:pc1], Ln accum to ll col."""
                na_t = na_fat[pr]
                e_t = fat.tile([128, B], bf16, name="e_t", bufs=2)
                nc.scalar.activation(e_t[:, pc0:pc1], na_t[:, pc0:pc1],
                                     AFT.Exp, scale=INV_T)
                l_t = fat.tile([128, B], bf16, name="l_t", bufs=2)
                nc.scalar.activation(l_t[:, pc0:pc1], e_t[:, pc0:pc1],
                                     AFT.Ln, bias=1.0,
                                     accum_out=ll_st[:, ll_col:ll_col + 1])

            piece_at = {}   # (r, n4) -> list of LL_PIECES entries
            piece_end = []  # issued after the loop
            for piece in LL_PIECES:
                w = LL_SCHED[piece[3]]
                if w is None:
                    piece_end.append(piece)
                else:
                    piece_at.setdefault(w, []).append(piece)

            for r in range(NR):
                na_fat[r] = fat.tile([128, B], bf16, name=f"na{r % 2}")
                for n4 in range(NN):
                    idx = r * NN + n4
                    c0 = n4 * CHUNK
                    psS = pp_s.tile([128, CHUNK], f32, name="psS")
                    for kc in range(KC):
                        for h in range(2):
                            nc.tensor.matmul(
                                psS[:, h * 512:(h + 1) * 512],
                                et_sb[kc][:, r * 128:(r + 1) * 128],
                                et_sb[kc][:, c0 + h * 512:c0 + (h + 1) * 512],
                                start=(kc == 0), stop=(kc == KC - 1))
                    psG = pp_g.tile([128, CHUNK], f32, name="psG")
                    for h in range(2):
                        nc.tensor.matmul(
                            psG[:, h * 512:(h + 1) * 512],
                            sfn_sb[:, r * 128:(r + 1) * 128],
                            sfn_sb[:, c0 + h * 512:c0 + (h + 1) * 512],
                            start=True, stop=True)

                    if (r, n4) in B_TILES:
                        # variant B: compare+scale on DVE, no ACT use
                        b2 = scratch.tile([128, CHUNK], bf16, name="b2")
                        nc.vector.tensor_scalar(
                            out=b2, in0=psG, scalar1=0.5, scalar2=2.0,
                            op0=ALU.is_gt, op1=ALU.mult,
                            accum_out=c_st[:, idx:idx + 1])
                        m_t = scratch.tile([128, CHUNK], bf16, name="m_t")
                        nc.vector.scalar_tensor_tensor(
                            out=m_t, in0=b2, scalar=-1.0, in1=psS,
                            op0=ALU.add, op1=ALU.mult,
                            accum_out=m_st[:, idx:idx + 1])
                    else:
                        # variant A: sign on ACT
                        sgn_t = scratch.tile([128, CHUNK], bf16, name="sgn_t")
                        nc.scalar.activation(sgn_t, psG, AFT.Sign,
                                             bias=neg_half,
                                             accum_out=c_st[:, idx:idx + 1])
                        m_t = scratch.tile([128, CHUNK], bf16, name="m_t")
                        nc.vector.scalar_tensor_tensor(
                            out=m_t, in0=sgn_t, scalar=1.0, in1=psS,
                            op0=ALU.mult, op1=ALU.mult,
                            accum_out=m_st[:, idx:idx + 1])
                    # na = min(-m, m) = -|m|, accum -> -AB (accumulating DVE
                    # ops always run at 1x, so one fused STT beats any split)
                    nc.vector.scalar_tensor_tensor(
                        out=na_fat[r][:, c0:c0 + CHUNK], in0=m_t, scalar=-1.0,
                        in1=m_t, op0=ALU.mult, op1=ALU.min,
                        accum_out=ab_st[:, idx:idx + 1])
                    for piece in piece_at.get((r, n4), []):
                        act_expln(*piece)
            for piece in piece_end:
                act_expln(*piece)

        nc.sync.dma_start(out=out_st, in_=st)

    orig = nc.to_json_bytes

    def patched():
        m = json.loads(orig())
        _split_multiwaits(m)
        return json.dumps(m).encode()

    nc.to_json_bytes = patched
    return nc


_NC_CACHE = None
last_run = None  # BassKernelResults of the most recent kernel() call


def _get_nc():
    global _NC_CACHE
    if _NC_CACHE is None:
        _NC_CACHE = _build_nc()
    return _NC_CACHE


def kernel(embeddings: np.ndarray, similarity_features: np.ndarray) -> np.ndarray:
    global last_run
    E = np.asarray(embeddings, dtype=np.float32)
    SF = np.asarray(similarity_features, dtype=np.float32)
    assert E.shape == (B, D) and SF.shape == (B, F)

    ET = E.T.astype(ml_dtypes.bfloat16)    # [D, B]
    fn = SF.astype(np.float64)
    fn = fn / np.maximum(np.linalg.norm(fn, axis=1, keepdims=True), 1e-12)
    SFNT = fn.T.astype(ml_dtypes.bfloat16)  # [F, B] normalized
    in_maps = []
    for c in range(NCORES):
        sh = c * RPC
        in_maps.append({
            "et": np.ascontiguousarray(np.roll(ET, -sh, axis=1)),
            "sfn": np.ascontiguousarray(np.roll(SFNT, -sh, axis=1)),
        })

    nc = _get_nc()
    res = run_bass_kernel_spmd(nc, in_maps, core_ids=list(range(NCORES)))
    last_run = res

    # host combine: [p, r*NN+n] stats -> per-row scalars (order within a core:
    # local row i = r*128 + p; global row = c*RPC + i, irrelevant to the sums)
    bce_num = np.zeros((NCORES, RPC), np.float64)
    pos_all = np.zeros((NCORES, RPC), np.float64)
    is_b = np.array([[(r, n) in B_TILES for n in range(NN)] for r in range(NR)])
    for c, r in enumerate(res.results):
        stt = r["out_st"].astype(np.float64)
        ab = -stt[:, AB_OFF:AB_OFF + NT].reshape(128, NR, NN).sum(axis=2)
        m = stt[:, M_OFF:M_OFF + NT].reshape(128, NR, NN).sum(axis=2)
        llp = stt[:, LL_OFF:LL_OFF + NLL]
        ll = np.zeros((128, NR))
        for (pr, _, _, col) in LL_PIECES:
            ll[:, pr] += llp[:, col]
        cc = stt[:, C_OFF:C_OFF + NT].reshape(128, NR, NN)
        # variant A columns hold sum(sgn) = 2*pos - 1024; variant B hold 2*pos
        pos = np.where(is_b[None, :, :], cc / 2.0, (cc + CHUNK) / 2.0).sum(axis=2)
        bce = 0.5 * INV_T * (ab - m) + ll            # [128, NR]
        bce_num[c] = bce.T.reshape(RPC)              # index r*128+p
        pos_all[c] = pos.T.reshape(RPC)

    bce_num = bce_num.reshape(-1)
    pos_all = pos_all.reshape(-1)
    row_loss = bce_num / np.float64(B - 1)
    pos_off = pos_all - 1.0                 # drop the diagonal positive
    neg_off = (B - 1) - pos_off
    valid = (pos_off >= 0.5) & (neg_off >= 0.5)
    num_valid = max(int(valid.sum()), 1)
    loss = np.float64(np.sum(np.where(valid, row_loss, 0.0))) / num_valid
    return np.float32(loss)


# revision 43
# speedup vs baseline: 1.0847x; 1.0486x over previous
"""Trainium2 Bass kernel for nn_ContrastiveLoss (B=4096, D=512, F=128), 8 NeuronCores.

Strategy (row-sharded, per sharding hint): core c owns rows [c*512, (c+1)*512).
Host passes each core a column-rolled, transposed copy of the inputs so the
core's own rows are always local columns 0:512 (static NEFF, no per-core code).
similarity_features are L2-normalized on HOST (trivial prep, removes the whole
on-device rsqrt chain and its startup serialization).

Per core, fully fused on device (S and tsim never touch HBM), per [128 x 1024]
tile (16 tiles = 4 row-blocks x 4 col-chunks):
  S_raw = E_loc @ E^T        (PE, bf16 -> fp32 PSUM, 4 k-chunks)
  G     = f_loc @ f^T        (PE)
  variant A (9 tiles, ACT+DVE):
    sgn = Sign(G - 0.5)              (ACT, accum -> Ssgn = 2C-1024)
    m   = sgn * S_raw                (DVE STT 1x, accum -> M)
  variant B (7 tiles, DVE-only; balances engine load):
    b2  = (G > 0.5) * 2              (DVE TS 1x, accum -> 2C)
    m   = (b2 - 1) * S_raw           (DVE STT 1x, accum -> M)  [b2-1 = sgn]
  both:
    na  = min(-m, m) = -|m|          (DVE STT, accum -> -AB; accumulating
                                      DVE ops always run 1x, so one fused
                                      STT beats any multi-pass split)
Per row-block (fat tiles amortize ACT's ~352-cyc per-instruction overhead;
the last row-block runs in two halves to shrink the pipeline tail):
  e = Exp(10 * na)                   (ACT [128,4096] bf16)
  l = Ln(e + 1)                      (ACT, accum -> LL)
Row BCE sum over all j (diagonal contributes exactly 0):
  sum_j softplus(-S*sgn) = 5*(AB - M) + LL      [T = 0.1 -> scale 10]
Host combines per-row stats: row_loss, validity, final scalar.

Sign/Exp/Ln/min/mult all live in the natural_log_exp_and_others ACT table set
-> no mid-kernel table switches. (Softplus LUT is absent in this build: the
softplus_and_others set's anchor is overlaid by act2.)

This walrus build caps sync waits at 1 per instruction; _split_multiwaits
legalizes the Tile-emitted BIR by hoisting extra waits onto single-wait Drains.
"""

import json
import ml_dtypes
import numpy as np
from contextlib import ExitStack

import concourse.bass as bass
import concourse.tile as tile
import concourse.mybir as mybir
from concourse.bass_utils import run_bass_kernel_spmd

f32 = mybir.dt.float32
bf16 = mybir.dt.bfloat16
AFT = mybir.ActivationFunctionType
ALU = mybir.AluOpType

B, D, F = 4096, 512, 128
NCORES = 8
RPC = B // NCORES          # 512 rows per core
NR = RPC // 128            # 4 row blocks of 128
CHUNK = 1024               # column chunk (2 PSUM banks)
NN = B // CHUNK            # 4 column chunks
NT = NR * NN               # 16 stat columns
KC = D // 128              # 4 contraction chunks
INV_T = 10.0               # 1/TEMPERATURE
# DVE-only tiles (variant B): 2 per row-block for r<3, 1 for r=3 -> 7 total.
# Balances ACT (Sign+Exp+Ln) against DVE (m/na/b2) engine time.
B_TILES = frozenset([(0, 1), (0, 3), (1, 1), (1, 3), (2, 1), (2, 3), (3, 1)])


def _split_multiwaits(m: dict) -> int:
    """Split >1-wait instructions into single-wait Drain chains (walrus cap)."""
    n_new = 0
    for fn in m["functions"]:
        for blk in fn["blocks"]:
            out = []
            for inst in blk["instructions"]:
                si = inst.get("sync_info") or {}
                ow = si.get("on_wait") or []
                if len(ow) > 1:
                    for w in ow[:-1]:
                        n_new += 1
                        out.append({
                            "debug": inst.get("debug", 0),
                            "engine": inst["engine"],
                            "ins": [], "outs": [],
                            "is_reset_sema": False,
                            "name": f"{inst['name']}-sw{n_new}",
                            "opcode": "Drain",
                            "sync_info": {"on_update": [], "on_wait": [w]},
                        })
                    si["on_wait"] = [ow[-1]]
                out.append(inst)
            blk["instructions"] = out
    return n_new


def _build_nc() -> bass.Bass:
    nc = bass.Bass("TRN2", target_bir_lowering=False, debug=False)
    et_d = nc.dram_tensor("et", [D, B], bf16, kind="ExternalInput").ap()
    sfn_d = nc.dram_tensor("sfn", [F, B], bf16, kind="ExternalInput").ap()
    out_ab = nc.dram_tensor("out_ab", [128, NT], f32, kind="ExternalOutput").ap()
    out_m = nc.dram_tensor("out_m", [128, NT], f32, kind="ExternalOutput").ap()
    # NR+1 columns: last row-block's Ln is split in two halves, each with its
    # own accum column (accum_out overwrites, it does not accumulate)
    out_ll = nc.dram_tensor("out_ll", [128, NR + 1], f32,
                            kind="ExternalOutput").ap()
    out_c = nc.dram_tensor("out_c", [128, NT], f32, kind="ExternalOutput").ap()

    with tile.TileContext(nc) as tc, ExitStack() as ctx:
        main = ctx.enter_context(tc.tile_pool(name="main", bufs=1))
        scratch = ctx.enter_context(tc.tile_pool(name="scratch", bufs=3))
        fat = ctx.enter_context(tc.tile_pool(name="fat", bufs=2))

        sfn_sb = main.tile([F, B], bf16, name="sfn_sb")
        for n4 in range(NN):
            c0 = n4 * CHUNK
            nc.sync.dma_start(out=sfn_sb[:, c0:c0 + CHUNK],
                              in_=sfn_d[:, c0:c0 + CHUNK])
        # column-piece DMAs: n4=0 pieces land first so tile (r,0) matmuls can
        # start while the rest of et streams in, and pieces spread over queues
        et_sb = [main.tile([128, B], bf16, name=f"et{kc}") for kc in range(KC)]
        for n4 in range(NN):
            for kc in range(KC):
                c0 = n4 * CHUNK
                nc.sync.dma_start(
                    out=et_sb[kc][:, c0:c0 + CHUNK],
                    in_=et_d[kc * 128:(kc + 1) * 128, c0:c0 + CHUNK])

        neg_half = main.tile([128, 1], f32, name="neg_half")
        nc.vector.memset(neg_half, -0.5)

        ab_st = main.tile([128, NT], f32, name="ab_st")
        m_st = main.tile([128, NT], f32, name="m_st")
        ll_st = main.tile([128, NR + 1], f32, name="ll_st")
        c_st = main.tile([128, NT], f32, name="c_st")

        # --- main loop over 16 tiles [128 rows x 1024 cols] ---
        with tc.tile_pool(name="pp_s", bufs=2, space="PSUM") as pp_s, \
             tc.tile_pool(name="pp_g", bufs=2, space="PSUM") as pp_g:
            na_fat = {}

            def act_rowblock(r):
                """Fat Exp+Ln over row-block r's na tile (split in 2 for the
                last row-block to shrink the pipeline tail)."""
                na_t = na_fat.pop(r)
                halves = 2 if r == NR - 1 else 1
                w = B // halves
                for h in range(halves):
                    e_t = fat.tile([128, B], bf16, name="e_t", bufs=2)
                    nc.scalar.activation(e_t[:, h * w:(h + 1) * w],
                                         na_t[:, h * w:(h + 1) * w],
                                         AFT.Exp, scale=INV_T)
                    l_t = fat.tile([128, B], bf16, name="l_t", bufs=2)
                    nc.scalar.activation(
                        l_t[:, h * w:(h + 1) * w], e_t[:, h * w:(h + 1) * w],
                        AFT.Ln, bias=1.0,
                        accum_out=ll_st[:, r + h:r + h + 1])

            for r in range(NR):
                na_fat[r] = fat.tile([128, B], bf16, name=f"na{r % 2}")
                for n4 in range(NN):
                    idx = r * NN + n4
                    c0 = n4 * CHUNK
                    psS = pp_s.tile([128, CHUNK], f32, name="psS")
                    for kc in range(KC):
                        for h in range(2):
                            nc.tensor.matmul(
                                psS[:, h * 512:(h + 1) * 512],
                                et_sb[kc][:, r * 128:(r + 1) * 128],
                                et_sb[kc][:, c0 + h * 512:c0 + (h + 1) * 512],
                                start=(kc == 0), stop=(kc == KC - 1))
                    psG = pp_g.tile([128, CHUNK], f32, name="psG")
                    for h in range(2):
                        nc.tensor.matmul(
                            psG[:, h * 512:(h + 1) * 512],
                            sfn_sb[:, r * 128:(r + 1) * 128],
                            sfn_sb[:, c0 + h * 512:c0 + (h + 1) * 512],
                            start=True, stop=True)

                    if (r, n4) in B_TILES:
                        # variant B: compare+scale on DVE, no ACT use
                        b2 = scratch.tile([128, CHUNK], bf16, name="b2")
                        nc.vector.tensor_scalar(
                            out=b2, in0=psG, scalar1=0.5, scalar2=2.0,
                            op0=ALU.is_gt, op1=ALU.mult,
                            accum_out=c_st[:, idx:idx + 1])
                        m_t = scratch.tile([128, CHUNK], bf16, name="m_t")
                        nc.vector.scalar_tensor_tensor(
                            out=m_t, in0=b2, scalar=-1.0, in1=psS,
                            op0=ALU.add, op1=ALU.mult,
                            accum_out=m_st[:, idx:idx + 1])
                    else:
                        # variant A: sign on ACT
                        sgn_t = scratch.tile([128, CHUNK], bf16, name="sgn_t")
                        nc.scalar.activation(sgn_t, psG, AFT.Sign,
                                             bias=neg_half,
                                             accum_out=c_st[:, idx:idx + 1])
                        m_t = scratch.tile([128, CHUNK], bf16, name="m_t")
                        nc.vector.scalar_tensor_tensor(
                            out=m_t, in0=sgn_t, scalar=1.0, in1=psS,
                            op0=ALU.mult, op1=ALU.mult,
                            accum_out=m_st[:, idx:idx + 1])
                    # na = min(-m, m) = -|m|; bf16 SBUF-only STT
                    nc.vector.scalar_tensor_tensor(
                        out=na_fat[r][:, c0:c0 + CHUNK], in0=m_t, scalar=-1.0,
                        in1=m_t, op0=ALU.mult, op1=ALU.min,
                        accum_out=ab_st[:, idx:idx + 1])
                    # interleave previous row-block's fat ACT ops between the
                    # per-tile Signs so the ACT queue never blocks on fresh na
                    if n4 == 1 and r > 0:
                        act_rowblock(r - 1)
            act_rowblock(NR - 1)

        nc.sync.dma_start(out=out_ab, in_=ab_st)
        nc.sync.dma_start(out=out_m, in_=m_st)
        nc.sync.dma_start(out=out_ll, in_=ll_st)
        nc.sync.dma_start(out=out_c, in_=c_st)

    orig = nc.to_json_bytes

    def patched():
        m = json.loads(orig())
        _split_multiwaits(m)
        return json.dumps(m).encode()

    nc.to_json_bytes = patched
    return nc


_NC_CACHE = None
last_run = None  # BassKernelResults of the most recent kernel() call


def _get_nc():
    global _NC_CACHE
    if _NC_CACHE is None:
        _NC_CACHE = _build_nc()
    return _NC_CACHE


def kernel(embeddings: np.ndarray, similarity_features: np.ndarray) -> np.ndarray:
    global last_run
    E = np.asarray(embeddings, dtype=np.float32)
    SF = np.asarray(similarity_features, dtype=np.float32)
    assert E.shape == (B, D) and SF.shape == (B, F)

    ET = E.T.astype(ml_dtypes.bfloat16)    # [D, B]
    fn = SF.astype(np.float64)
    fn = fn / np.maximum(np.linalg.norm(fn, axis=1, keepdims=True), 1e-12)
    SFNT = fn.T.astype(ml_dtypes.bfloat16)  # [F, B] normalized
    in_maps = []
    for c in range(NCORES):
        sh = c * RPC
        in_maps.append({
            "et": np.ascontiguousarray(np.roll(ET, -sh, axis=1)),
            "sfn": np.ascontiguousarray(np.roll(SFNT, -sh, axis=1)),
        })

    nc = _get_nc()
    res = run_bass_kernel_spmd(nc, in_maps, core_ids=list(range(NCORES)))
    last_run = res

    # host combine: [p, r*NN+n] stats -> per-row scalars (order within a core:
    # local row i = r*128 + p; global row = c*RPC + i, irrelevant to the sums)
    bce_num = np.zeros((NCORES, RPC), np.float64)
    pos_all = np.zeros((NCORES, RPC), np.float64)
    is_b = np.array([[(r, n) in B_TILES for n in range(NN)] for r in range(NR)])
    for c, r in enumerate(res.results):
        ab = -r["out_ab"].astype(np.float64).reshape(128, NR, NN).sum(axis=2)
        m = r["out_m"].astype(np.float64).reshape(128, NR, NN).sum(axis=2)
        ll = r["out_ll"].astype(np.float64)          # [128, NR+1]
        ll = np.concatenate(
            [ll[:, :NR - 1], (ll[:, NR - 1] + ll[:, NR])[:, None]], axis=1)
        cc = r["out_c"].astype(np.float64).reshape(128, NR, NN)
        # variant A columns hold sum(sgn) = 2*pos - 1024; variant B hold 2*pos
        pos = np.where(is_b[None, :, :], cc / 2.0, (cc + CHUNK) / 2.0).sum(axis=2)
        bce = 0.5 * INV_T * (ab - m) + ll            # [128, NR]
        bce_num[c] = bce.T.reshape(RPC)              # index r*128+p
        pos_all[c] = pos.T.reshape(RPC)

    bce_num = bce_num.reshape(-1)
    pos_all = pos_all.reshape(-1)
    row_loss = bce_num / np.float64(B - 1)
    pos_off = pos_all - 1.0                 # drop the diagonal positive
    neg_off = (B - 1) - pos_off
    valid = (pos_off >= 0.5) & (neg_off >= 0.5)
    num_valid = max(int(valid.sum()), 1)
    loss = np.float64(np.sum(np.where(valid, row_loss, 0.0))) / num_valid
    return np.float32(loss)
